# revision 11
# baseline (speedup 1.0000x reference)
"""Trainium2 Bass kernel for nn_Encoder (mLSTM encoder with boundary detector).

Strategy: feature-parallel across 8 NeuronCores.
- The x-dependent projections (Wih1, Wmx1, BD x-dot) are precomputed up front
  as large efficient matmuls (the embedding gather + transpose is done on the
  host; XT tiles are streamed in).
- The 512-step time recurrence runs with every weight matrix feature-sharded
  8 ways and resident in SBUF; activations are exchanged per step with
  AllGather collectives (3 per step: merged [h2T|m1T], [x2T|h1T|pdot], [m2T]).
- Matmuls use float32r (fp32-grade precision at bf16-rate when the moving
  free dim >= 256). All elementwise state math is fp32; the boundary-detector
  dot products are computed in fp32/f64 so the p>0.5 threshold matches the
  reference as closely as possible.
"""

import sys
import os
import numpy as np

for _p in ("/opt/trn_rl_repo", "/root/.axon_site/_ro/trn_rl_repo"):
    if os.path.isdir(_p) and _p not in sys.path:
        sys.path.insert(0, _p)

import concourse.bacc as bacc
import concourse.mybir as mybir
from concourse.tile import TileContext
from concourse.bass_utils import run_bass_kernel_spmd
from concourse.masks import make_identity

F32 = mybir.dt.float32
F32R = mybir.dt.float32r

V, E, H, MID, S, B = 32000, 512, 1024, 512, 512, 128
XV, XE = 64, 64
FEAT = E + XE            # 576
KP = 640                 # FEAT + 1 (ones) zero-padded to 5*128
NC = 8                   # cores
HS = H // NC             # 128 features per core
GS = 4 * HS              # 512 gate-features per core

STEPS = int(os.environ.get("BASS_ENC_STEPS", S))



import bass_rust


def _blocks_ap(dram, col0, ncols, nblk=8, blk=128):
    """3D dram AP over [nblk] row-blocks: out (p=blk, nblk, ncols)."""
    ap = dram[0:blk, col0:col0 + ncols]
    row = dram.shape[1]
    return bass_rust.AP(ap.tensor, ap.offset,
                        [[row, blk], [row * blk, nblk], [1, ncols]])


def _gate_rows(j):
    """Row indices of the 4 gates for core j, reordered [i, f, o, g]."""
    r = np.arange(HS) + HS * j
    return np.concatenate([r, H + r, 3 * H + r, 2 * H + r])


def build_nc(steps=STEPS):
    nc = bacc.Bacc(None, target_bir_lowering=False)

    xt_e = nc.declare_dram_parameter("xt", [S, 5, 128, 128], F32R, isOutput=False)
    wpre_e = nc.declare_dram_parameter("wpre", [5, 128, KP], F32R, isOutput=False)
    whh1_e = nc.declare_dram_parameter("whh1c", [8, 128, GS], F32R, isOutput=False)
    wih2_e = nc.declare_dram_parameter("wih2c", [8, 128, GS], F32R, isOutput=False)
    whh2_e = nc.declare_dram_parameter("whh2c", [8, 128, GS], F32R, isOutput=False)
    wmh1_e = nc.declare_dram_parameter("wmh1s", [8, 128, 128], F32R, isOutput=False)
    wmh2_e = nc.declare_dram_parameter("wmh2s", [8, 128, 128], F32R, isOutput=False)
    wmx2_e = nc.declare_dram_parameter("wmx2s", [8, 128, 128], F32R, isOutput=False)
    b2bc_e = nc.declare_dram_parameter("b2bc", [128, GS], F32, isOutput=False)
    whbc_e = nc.declare_dram_parameter("whbc", [128, 128], F32, isOutput=False)
    xdot_e = nc.declare_dram_parameter("xdot", [128, S], F32, isOutput=False)

    outs_e = nc.declare_dram_parameter("outs_p", [S, 128, HS], F32, isOutput=True)
    h2f_e = nc.declare_dram_parameter("h2f_p", [128, HS], F32, isOutput=True)
    c2f_e = nc.declare_dram_parameter("c2f_p", [128, HS], F32, isOutput=True)
    flags_e = nc.declare_dram_parameter("flags_o", [1, S], F32, isOutput=True)

    z1init_d = nc.dram_tensor("z1init_d", [S, 128, GS], F32)
    auxT_d = nc.dram_tensor("auxT_d", [S, 128, 128], F32)

    # AllGather bounce buffers (double-buffered). Gather concats on the
    # partition axis: out is [8*128, payload_cols].
    agA_in = [nc.dram_tensor(f"agA_in{i}", [128, 128], F32) for i in range(2)]
    agA_out = [nc.dram_tensor(f"agA_out{i}", [8 * 128, 128], F32, addr_space="Shared")
               for i in range(2)]
    agB_in = [nc.dram_tensor(f"agB_in{i}", [128, 385], F32) for i in range(2)]
    agB_out = [nc.dram_tensor(f"agB_out{i}", [8 * 128, 385], F32, addr_space="Shared")
               for i in range(2)]
    agC_in = [nc.dram_tensor(f"agC_in{i}", [128, 128], F32) for i in range(2)]
    agC_out = [nc.dram_tensor(f"agC_out{i}", [8 * 128, 128], F32, addr_space="Shared")
               for i in range(2)]

    AG = mybir.AluOpType.bypass
    RG = [list(range(NC))]

    def sig(out, in_):
        nc.scalar.activation(out, in_, mybir.ActivationFunctionType.Sigmoid)

    def tanh(out, in_):
        nc.scalar.activation(out, in_, mybir.ActivationFunctionType.Tanh)

    def tt(out, a, b, op):
        nc.vector.tensor_tensor(out, a, b, op)

    MUL = mybir.AluOpType.mult
    ADD = mybir.AluOpType.add
    SUB = mybir.AluOpType.subtract

    with TileContext(nc) as tc:
        with tc.tile_pool(name="persist", bufs=1) as pp:
            wpre_sb = pp.tile([128, 5 * KP], F32R)
            whh1_sb = pp.tile([128, 8 * GS], F32R)
            wih2_sb = pp.tile([128, 8 * GS], F32R)
            whh2_sb = pp.tile([128, 8 * GS], F32R)
            wmh1_sb = pp.tile([128, 8 * 128], F32R)
            wmh2_sb = pp.tile([128, 8 * 128], F32R)
            wmx2_sb = pp.tile([128, 8 * 128], F32R)
            b2bc_sb = pp.tile([128, GS], F32)
            whbc_sb = pp.tile([128, 128], F32)
            xdot_sb = pp.tile([128, S], F32)
            flags_sb = pp.tile([1, S], F32)
            c1_sb = pp.tile([128, HS], F32)
            c2_sb = pp.tile([128, HS], F32)
            h2bm_sb = pp.tile([128, HS], F32)
            s_pp = pp.tile([128, 1], F32)       # current s (batch-major)
            oms_pp = pp.tile([128, 1], F32)     # 1 - s
            ppsum_pp = pp.tile([128, 1], F32)   # sum of pdot partials (prev step)
            zero128 = pp.tile([128, 128], F32)
            ident = pp.tile([128, 128], F32)
            make_identity(nc, ident[:, :])

            for k in range(5):
                nc.sync.dma_start(out=wpre_sb[:, k * KP:(k + 1) * KP], in_=wpre_e[k])
            for k in range(8):
                nc.sync.dma_start(out=whh1_sb[:, k * GS:(k + 1) * GS], in_=whh1_e[k])
                nc.sync.dma_start(out=wih2_sb[:, k * GS:(k + 1) * GS], in_=wih2_e[k])
                nc.sync.dma_start(out=whh2_sb[:, k * GS:(k + 1) * GS], in_=whh2_e[k])
                nc.sync.dma_start(out=wmh1_sb[:, k * 128:(k + 1) * 128], in_=wmh1_e[k])
                nc.sync.dma_start(out=wmh2_sb[:, k * 128:(k + 1) * 128], in_=wmh2_e[k])
                nc.sync.dma_start(out=wmx2_sb[:, k * 128:(k + 1) * 128], in_=wmx2_e[k])
            nc.sync.dma_start(out=b2bc_sb[:, :], in_=b2bc_e[:, :])
            nc.sync.dma_start(out=whbc_sb[:, :], in_=whbc_e[:, :])
            nc.sync.dma_start(out=xdot_sb[:, :], in_=xdot_e[:, :])
            nc.vector.memset(flags_sb[:, :], 0.0)
            nc.vector.memset(c1_sb[:, :], 0.0)
            nc.vector.memset(c2_sb[:, :], 0.0)
            nc.vector.memset(h2bm_sb[:, :], 0.0)
            nc.vector.memset(s_pp[:, :], 0.0)
            nc.vector.memset(oms_pp[:, :], 1.0)
            nc.vector.memset(ppsum_pp[:, :], 0.0)
            nc.vector.memset(zero128[:, :], 0.0)

            # ---------------- Phase P: x-projections ----------------
            with tc.tile_pool(name="pre_sb", bufs=3) as pre, \
                 tc.tile_pool(name="pre_ps", bufs=2, space="PSUM") as prep:
                for t in range(steps):
                    xt_sb = pre.tile([128, 5 * 128], F32R, tag="xt")
                    for k in range(5):
                        nc.sync.dma_start(out=xt_sb[:, k * 128:(k + 1) * 128],
                                          in_=xt_e[t, k])
                    z1p = prep.tile([128, GS], F32, tag="z1p")
                    for k in range(5):
                        nc.tensor.matmul(z1p[:, :], xt_sb[:, k * 128:(k + 1) * 128],
                                         wpre_sb[:, k * KP:k * KP + GS],
                                         start=(k == 0), stop=(k == 4))
                    auxp = prep.tile([128, 128], F32, tag="auxp")
                    for k in range(5):
                        nc.tensor.matmul(auxp[:, :], xt_sb[:, k * 128:(k + 1) * 128],
                                         wpre_sb[:, k * KP + GS:(k + 1) * KP],
                                         start=(k == 0), stop=(k == 4))
                    z1c = pre.tile([128, GS], F32, tag="z1c")
                    nc.vector.tensor_copy(z1c[:, :], z1p[:, :])
                    nc.sync.dma_start(out=z1init_d[t], in_=z1c[:, :])
                    auxc = pre.tile([128, 128], F32, tag="auxc")
                    nc.vector.tensor_copy(auxc[:, :], auxp[:, :])
                    auxTp = prep.tile([128, 128], F32, tag="auxTp")
                    nc.tensor.transpose(auxTp[:, :], auxc[:, :], ident[:, :])
                    auxT = pre.tile([128, 128], F32, tag="auxT")
                    nc.vector.tensor_copy(auxT[:, :], auxTp[:, :])
                    nc.sync.dma_start(out=auxT_d[t], in_=auxT[:, :])

            # ---------------- Phase R: recurrence ----------------
            with tc.tile_pool(name="st_sb", bufs=3) as sp, \
                 tc.tile_pool(name="st_gath", bufs=2) as gp, \
                 tc.tile_pool(name="st_z", bufs=2, space="PSUM") as zp, \
                 tc.tile_pool(name="st_z2", bufs=2, space="PSUM") as zp2, \
                 tc.tile_pool(name="st_ps", bufs=4, space="PSUM") as smp:

                # initial gathered state (zeros)
                h1T_r = gp.tile([128, 8 * 128], F32R, tag="h1T")
                x2T_r = gp.tile([128, 8 * 128], F32R, tag="x2T")
                for buf in (h1T_r, x2T_r):
                    nc.vector.memset(buf[:, :].bitcast(F32), 0.0)

                h2T_pending = None  # h2T slice of t-1 (rides AG-B of step t)

                for t in range(steps):
                    pb = t % 2

                    # -- s(t) from xdot + (1-s_prev) * ppsum_prev
                    ptmp = sp.tile([128, 1], F32, tag="ptmp")
                    tt(ptmp[:, :], ppsum_pp[:, :], oms_pp[:, :], MUL)
                    tt(ptmp[:, :], ptmp[:, :], xdot_sb[:, t:t + 1], ADD)
                    nc.vector.tensor_scalar(s_pp[:, :], ptmp[:, :], 0.0, None,
                                            mybir.AluOpType.is_gt)
                    nc.vector.tensor_scalar(oms_pp[:, :], s_pp[:, :], -1.0, 1.0,
                                            MUL, ADD)
                    nc.vector.tensor_copy(flags_sb[:, t:t + 1], s_pp[0:1, 0:1])
                    flag_pp = sp.tile([128, 1], F32, tag="flag")
                    nc.gpsimd.partition_broadcast(flag_pp[:, :], s_pp[0:1, 0:1])
                    omf_pp = sp.tile([128, 1], F32, tag="omf")
                    nc.vector.tensor_scalar(omf_pp[:, :], flag_pp[:, :], -1.0, 1.0,
                                            MUL, ADD)

                    # -- hop1: m1h = (Wmh1 h1)^T slice, h2m = (Wmh2 h2)^T slice
                    aux_t = sp.tile([128, 128], F32, tag="aux")
                    nc.sync.dma_start(out=aux_t[:, :], in_=auxT_d[t])
                    m1h_ps = smp.tile([128, 128], F32, tag="sm")
                    for k in range(8):
                        nc.tensor.matmul(m1h_ps[:, :], wmh1_sb[:, k * 128:(k + 1) * 128],
                                         h1T_r[:, k * 128:(k + 1) * 128],
                                         start=(k == 0), stop=(k == 7))
                    payA = sp.tile([128, 128], F32, tag="payA")
                    tt(payA[:, :], m1h_ps[:, :], aux_t[:, :], MUL)
                    nc.sync.dma_start(out=agA_in[pb][:, :], in_=payA[:, :])
                    nc.gpsimd.collective_compute(
                        "AllGather", AG, replica_groups=RG,
                        ins=[agA_in[pb][:].opt()], outs=[agA_out[pb][:].opt()])
                    m1T_r = gp.tile([128, 8 * 128], F32R, tag="m1T")
                    nc.sync.dma_start(out=m1T_r[:, :],
                                      in_=_blocks_ap(agA_out[pb], 0, 128).bitcast(F32R))

                    # -- hop2: z1 = z1init + m1 @ Whh1c^T
                    z1init_t = sp.tile([128, GS], F32, tag="z1init")
                    nc.sync.dma_start(out=z1init_t[:, :], in_=z1init_d[t])
                    z1_ps = zp.tile([128, GS], F32, tag="z1")
                    for k in range(8):
                        nc.tensor.matmul(z1_ps[:, :], m1T_r[:, k * 128:(k + 1) * 128],
                                         whh1_sb[:, k * GS:(k + 1) * GS],
                                         start=(k == 0), stop=(k == 7))
                    z1s = sp.tile([128, GS], F32, tag="z1s")
                    tt(z1s[:, :], z1_ps[:, :], z1init_t[:, :], ADD)
                    # gates [i | f | o | g]
                    gsig = sp.tile([128, 384], F32, tag="gsig")
                    sig(gsig[:, :], z1s[:, 0:384])
                    gtan = sp.tile([128, 128], F32, tag="gtan")
                    tanh(gtan[:, :], z1s[:, 384:512])
                    t1 = sp.tile([128, HS], F32, tag="t1")
                    tt(t1[:, :], gsig[:, 0:128], gtan[:, :], MUL)
                    t2 = sp.tile([128, HS], F32, tag="t2")
                    tt(t2[:, :], gsig[:, 128:256], c1_sb[:, :], MUL)
                    c1n = sp.tile([128, HS], F32, tag="c1n")
                    tt(c1n[:, :], t1[:, :], t2[:, :], ADD)
                    tc1 = sp.tile([128, HS], F32, tag="tc1")
                    tanh(tc1[:, :], c1n[:, :])
                    h1n = sp.tile([128, HS], F32, tag="h1n")
                    tt(h1n[:, :], gsig[:, 256:384], tc1[:, :], MUL)
                    nc.vector.tensor_scalar(c1_sb[:, :], c1n[:, :], oms_pp[:, :],
                                            None, MUL)
                    # pdot partial (fp32): sum_f h1n * whbc
                    pd = sp.tile([128, HS], F32, tag="pd")
                    tt(pd[:, :], h1n[:, :], whbc_sb[:, :], MUL)
                    ppart = sp.tile([128, 1], F32, tag="ppart")
                    nc.vector.tensor_reduce(ppart[:, :], pd[:, :],
                                            mybir.AxisListType.X, mybir.AluOpType.add)
                    # x2 / new h1 (batch-major), then transpose
                    x2bm = sp.tile([128, HS], F32, tag="x2bm")
                    nc.vector.tensor_scalar(x2bm[:, :], h1n[:, :], s_pp[:, :], None, MUL)
                    h1bm = sp.tile([128, HS], F32, tag="h1bm")
                    tt(h1bm[:, :], h1n[:, :], x2bm[:, :], SUB)
                    x2T_ps = smp.tile([128, 128], F32, tag="sm")
                    nc.tensor.transpose(x2T_ps[:, :], x2bm[:, :], ident[:, :])
                    h1T_ps = smp.tile([128, 128], F32, tag="sm")
                    nc.tensor.transpose(h1T_ps[:, :], h1bm[:, :], ident[:, :])
                    payB = sp.tile([128, 385], F32, tag="payB")
                    nc.vector.tensor_copy(payB[:, 0:128], x2T_ps[:, :])
                    nc.vector.tensor_copy(payB[:, 128:256], h1T_ps[:, :])
                    if h2T_pending is None:
                        nc.vector.tensor_copy(payB[:, 256:384], zero128[:, :])
                    else:
                        nc.vector.tensor_copy(payB[:, 256:384], h2T_pending[:, :])
                    nc.vector.tensor_copy(payB[:, 384:385], ppart[:, :])
                    nc.sync.dma_start(out=agB_in[pb][:, :], in_=payB[:, :])
                    nc.gpsimd.collective_compute(
                        "AllGather", AG, replica_groups=RG,
                        ins=[agB_in[pb][:].opt()], outs=[agB_out[pb][:].opt()])
                    x2T_r = gp.tile([128, 8 * 128], F32R, tag="x2T")
                    h1T_r = gp.tile([128, 8 * 128], F32R, tag="h1T")
                    h2T_r = gp.tile([128, 8 * 128], F32R, tag="h2T")
                    nc.sync.dma_start(out=x2T_r[:, :],
                                      in_=_blocks_ap(agB_out[pb], 0, 128).bitcast(F32R))
                    nc.sync.dma_start(out=h1T_r[:, :],
                                      in_=_blocks_ap(agB_out[pb], 128, 128).bitcast(F32R))
                    nc.sync.dma_start(out=h2T_r[:, :],
                                      in_=_blocks_ap(agB_out[pb], 256, 128).bitcast(F32R))
                    pp8 = sp.tile([128, 8], F32, tag="pp8")
                    nc.sync.dma_start(
                        out=pp8[:, :],
                        in_=agB_out[pb][:, 384:385].rearrange("(r p) one -> p (r one)", r=8))
                    nc.vector.tensor_reduce(ppsum_pp[:, :], pp8[:, :],
                                            mybir.AxisListType.X, mybir.AluOpType.add)

                    # -- hop3: m2 factors first (feed AG-C), then z2 x2-part
                    m2x_ps = smp.tile([128, 128], F32, tag="sm")
                    for k in range(8):
                        nc.tensor.matmul(m2x_ps[:, :], wmx2_sb[:, k * 128:(k + 1) * 128],
                                         x2T_r[:, k * 128:(k + 1) * 128],
                                         start=(k == 0), stop=(k == 7))
                    h2m_ps = smp.tile([128, 128], F32, tag="sm")
                    for k in range(8):
                        nc.tensor.matmul(h2m_ps[:, :], wmh2_sb[:, k * 128:(k + 1) * 128],
                                         h2T_r[:, k * 128:(k + 1) * 128],
                                         start=(k == 0), stop=(k == 7))
                    h2m_sb = sp.tile([128, 128], F32, tag="h2m")
                    nc.vector.tensor_copy(h2m_sb[:, :], h2m_ps[:, :])
                    payC = sp.tile([128, 128], F32, tag="payC")
                    tt(payC[:, :], m2x_ps[:, :], h2m_sb[:, :], MUL)
                    nc.sync.dma_start(out=agC_in[pb][:, :], in_=payC[:, :])
                    nc.gpsimd.collective_compute(
                        "AllGather", AG, replica_groups=RG,
                        ins=[agC_in[pb][:].opt()], outs=[agC_out[pb][:].opt()])
                    z2_ps = zp2.tile([128, GS], F32, tag="z2")
                    for k in range(8):
                        nc.tensor.matmul(z2_ps[:, :], x2T_r[:, k * 128:(k + 1) * 128],
                                         wih2_sb[:, k * GS:(k + 1) * GS],
                                         start=(k == 0), stop=False)
                    m2T_r = gp.tile([128, 8 * 128], F32R, tag="m2T")
                    nc.sync.dma_start(out=m2T_r[:, :],
                                      in_=_blocks_ap(agC_out[pb], 0, 128).bitcast(F32R))

                    # -- hop4: z2 += m2 @ Whh2c^T
                    for k in range(8):
                        nc.tensor.matmul(z2_ps[:, :], m2T_r[:, k * 128:(k + 1) * 128],
                                         whh2_sb[:, k * GS:(k + 1) * GS],
                                         start=False, stop=(k == 7))
                    z2s = sp.tile([128, GS], F32, tag="z2s")
                    tt(z2s[:, :], z2_ps[:, :], b2bc_sb[:, :], ADD)
                    gsig2 = sp.tile([128, 384], F32, tag="gsig2")
                    sig(gsig2[:, :], z2s[:, 0:384])
                    gtan2 = sp.tile([128, 128], F32, tag="gtan2")
                    tanh(gtan2[:, :], z2s[:, 384:512])
                    t1b = sp.tile([128, HS], F32, tag="t1b")
                    tt(t1b[:, :], gsig2[:, 0:128], gtan2[:, :], MUL)
                    t2b = sp.tile([128, HS], F32, tag="t2b")
                    tt(t2b[:, :], gsig2[:, 128:256], c2_sb[:, :], MUL)
                    c2n = sp.tile([128, HS], F32, tag="c2n")
                    tt(c2n[:, :], t1b[:, :], t2b[:, :], ADD)
                    tc2 = sp.tile([128, HS], F32, tag="tc2")
                    tanh(tc2[:, :], c2n[:, :])
                    h2n = sp.tile([128, HS], F32, tag="h2n")
                    tt(h2n[:, :], gsig2[:, 256:384], tc2[:, :], MUL)
                    # conditional (flag) updates of c2, h2
                    nc.vector.tensor_scalar(c2_sb[:, :], c2_sb[:, :], omf_pp[:, :],
                                            None, MUL)
                    nc.vector.scalar_tensor_tensor(c2_sb[:, :], c2n[:, :],
                                                   flag_pp[:, :], c2_sb[:, :], MUL, ADD)
                    nc.vector.tensor_scalar(h2bm_sb[:, :], h2bm_sb[:, :], omf_pp[:, :],
                                            None, MUL)
                    nc.vector.scalar_tensor_tensor(h2bm_sb[:, :], h2n[:, :],
                                                   flag_pp[:, :], h2bm_sb[:, :], MUL, ADD)
                    # write step output
                    ob = sp.tile([128, HS], F32, tag="ob")
                    nc.vector.tensor_copy(ob[:, :], h2bm_sb[:, :])
                    nc.sync.dma_start(out=outs_e[t], in_=ob[:, :])
                    # h2T slice payload for next step's merged AG
                    h2T_ps = smp.tile([128, 128], F32, tag="sm")
                    nc.tensor.transpose(h2T_ps[:, :], h2bm_sb[:, :], ident[:, :])
                    h2T_pending = sp.tile([128, 128], F32, tag="h2Tpend")
                    nc.vector.tensor_copy(h2T_pending[:, :], h2T_ps[:, :])

                # epilogue
                nc.sync.dma_start(out=h2f_e[:, :], in_=h2bm_sb[:, :])
                c2o = sp.tile([128, HS], F32, tag="c2o")
                nc.vector.tensor_copy(c2o[:, :], c2_sb[:, :])
                nc.sync.dma_start(out=c2f_e[:, :], in_=c2o[:, :])
                nc.sync.dma_start(out=flags_e[:, :], in_=flags_sb[:, :])

    nc.compile()
    return nc


def _prep_inputs(inputs):
    """Host-side preparation of per-core input dicts."""
    f32 = np.float32
    ids_w = np.asarray(inputs["encoder_inputs"]).astype(np.int64)
    ids_x = np.asarray(inputs["encoder_extra_inputs"]).astype(np.int64)
    word_emb = np.asarray(inputs["word_emb"], dtype=f32)
    extra_emb = np.asarray(inputs["extra_emb"], dtype=f32)
    Wsi = np.asarray(inputs["Wsi"], dtype=f32)
    Wsh = np.asarray(inputs["Wsh"], dtype=f32)
    b_bd = np.asarray(inputs["b_bd"], dtype=f32)
    vs = np.asarray(inputs["vs"], dtype=f32)

    embeds = np.concatenate([word_emb[ids_w], extra_emb[ids_x]], axis=-1)  # [B,S,576]

    # XT tiles: [S, 5, 128, 128] with K = [x features 0:576 | ones | zeros]
    Xp = np.zeros((B, S, KP), dtype=f32)
    Xp[:, :, :FEAT] = embeds
    Xp[:, :, FEAT] = 1.0
    xt = np.ascontiguousarray(
        Xp.transpose(1, 2, 0).reshape(S, 5, 128, 128))

    # BD folded vectors (f64 for max fidelity at the p>0.5 threshold)
    u1 = (vs.astype(np.float64) @ Wsi.astype(np.float64))[0]      # [576]
    wh = (vs.astype(np.float64) @ Wsh.astype(np.float64))[0]      # [1024]
    c0 = float(vs.astype(np.float64)[0] @ b_bd.astype(np.float64))
    xdot = (embeds.astype(np.float64) @ u1 + c0).astype(f32)      # [B, S]

    in_maps = []
    for j in range(NC):
        gr = _gate_rows(j)
        w1 = np.asarray(inputs["Wih1"], dtype=f32)[gr]       # [512, 576]
        b1 = np.asarray(inputs["b1"], dtype=f32)[gr]         # [512]
        wmx1 = np.asarray(inputs["Wmx1"], dtype=f32)[HS * j:HS * (j + 1)]  # [128,576]
        wpre = np.zeros((5, 128, KP), dtype=f32)
        wcat = np.zeros((KP, KP), dtype=f32)  # rows: K (padded feat), cols: [z1 512 | aux 128]
        wcat[:FEAT, :GS] = w1.T
        wcat[FEAT, :GS] = b1
        wcat[:FEAT, GS:GS + 128] = wmx1.T
        wpre[:, :, :] = wcat.reshape(5, 128, KP)

        def ktiles(Wc, n):
            # rhs tiles [8, 128, n] from [n, 1024] matrix: block k = Wc[:, 128k:128k+128].T
            return np.ascontiguousarray(
                Wc.reshape(n, 8, 128).transpose(1, 2, 0))

        whh1c = ktiles(np.asarray(inputs["Whh1"], dtype=f32)[gr], GS)
        wih2c = ktiles(np.asarray(inputs["Wih2"], dtype=f32)[gr], GS)
        whh2c = ktiles(np.asarray(inputs["Whh2"], dtype=f32)[gr], GS)

        def stiles(W):
            # stationary tiles [8, 128, 128]: block k = W[128j:128j+128, 128k:128k+128].T
            Wj = W[HS * j:HS * (j + 1)]           # [128, 1024]
            return np.ascontiguousarray(Wj.reshape(128, 8, 128).transpose(1, 2, 0))

        wmh1s = stiles(np.asarray(inputs["Wmh1"], dtype=f32))
        wmh2s = stiles(np.asarray(inputs["Wmh2"], dtype=f32))
        wmx2s = stiles(np.asarray(inputs["Wmx2"], dtype=f32))

        b2 = np.asarray(inputs["b2"], dtype=f32)[gr]
        b2bc = np.broadcast_to(b2, (128, GS)).copy()
        whbc = np.broadcast_to(wh[HS * j:HS * (j + 1)].astype(f32), (128, 128)).copy()

        in_maps.append({
            "xt": xt, "wpre": wpre,
            "whh1c": whh1c, "wih2c": wih2c, "whh2c": whh2c,
            "wmh1s": wmh1s, "wmh2s": wmh2s, "wmx2s": wmx2s,
            "b2bc": b2bc, "whbc": whbc, "xdot": np.ascontiguousarray(xdot),
        })
    return in_maps


_NC_CACHE = {}


def kernel(**inputs):
    steps = STEPS
    if steps not in _NC_CACHE:
        _NC_CACHE[steps] = build_nc(steps)
    nc = _NC_CACHE[steps]
    in_maps = _prep_inputs(inputs)
    import time as _time
    _t0 = _time.time()
    res = run_bass_kernel_spmd(nc, in_maps, core_ids=list(range(NC)),
                               trace=bool(int(os.environ.get("BASS_ENC_TRACE", "0"))))
    kernel.last_run_wall = _time.time() - _t0
    if int(os.environ.get("BASS_ENC_TIME", "0")):
        _t1 = _time.time()
        res = run_bass_kernel_spmd(nc, in_maps, core_ids=list(range(NC)))
        kernel.last_run_wall2 = _time.time() - _t1
        print(f"[kernel] run1={kernel.last_run_wall:.2f}s run2={kernel.last_run_wall2:.2f}s")
    kernel.last_results = res
    outs = np.zeros((B, S, H), dtype=np.float32)
    for j in range(NC):
        r = res.results[j]
        outs[:, :, HS * j:HS * (j + 1)] = r["outs_p"].transpose(1, 0, 2)
    h2f = np.concatenate([res.results[j]["h2f_p"] for j in range(NC)], axis=1)[None]
    c2f = np.concatenate([res.results[j]["c2f_p"] for j in range(NC)], axis=1)[None]
    flags = res.results[0]["flags_o"][0]
    return outs, (h2f, c2f), flags
